# revision 15
# baseline (speedup 1.0000x reference)
"""Trainium2 Bass kernel for nn_Attn_45423574123081 (sparse_attention).

Computes, for inputs enc [B=32, L=1024, D=64], W [64, 64], b [64]:
    energy = enc @ W.T + b                       # [B, L, D]
    scores[t, b, j] = energy[b, j] . enc[b, t]   # [T=1024, B, L]
    scores[t, :, t] = 0
    out = softmax(scores, axis=-1)

Sharding: data-parallel over batch; 4 batches per core on 8 NeuronCores.

Key algebraic simplification: scores[t, :, j] = (E W^T E^T)[t, j] + c_t
with c_t = b . E[t] constant along the softmax axis j.  Softmax is
invariant to a per-row constant, so the bias only matters through the
zeroed diagonal: softmax(row with diag forced to 0) equals softmax of
the UNBIASED row with the diagonal forced to -c_t.  So G = E @ W^T
(no bias), and the diagonal write becomes copy_predicated(-E.b).

Precision strategy: matmuls run in fp16.  E is cast once to fp16
(~5e-4 score error).  W and G use an exact fp16 hi/lo split
(x = hi + lo), with the G halves stacked along the PE contraction axis
(K <= 128 is free on the systolic array), so each 512-wide PSUM bank of
S needs exactly ONE K=128 matmul:
    [E16; E16] @ [Glo; Ghi] = E16 @ G
Cross-partition placements (E16 row duplication, Ghi upper half) are
small SBUF->SBUF DMAs on the otherwise-idle GpSimd SWDGE queue, so they
never sit behind the megabyte output DMAs.

The emission is software-pipelined in two phases per batch (operand
prep A: load/cast/transpose/duplicate; prep B: G + E.b) interleaved
into the previous batch's S-block stream, so the per-engine in-order
queues never stall on a serial prep chain.

Softmax skips the max-subtraction: |scores| <= ~50 for this problem
family and exp(50) is far below f32 overflow.  exp runs on ScalarE with
a fused per-row accumulated sum; normalize is a per-partition scalar
multiply on VectorE.
"""

import numpy as np

_B, _L, _D, _T = 32, 1024, 64, 1024
_N_CORES = 8
_BPC = _B // _N_CORES  # batches per core

_compiled_nc = None


def _build():
    global _compiled_nc
    if _compiled_nc is not None:
        return _compiled_nc

    import concourse.bacc as bacc
    import concourse.mybir as mybir
    from concourse import tile, masks

    dt = mybir.dt
    AF = mybir.ActivationFunctionType

    nc = bacc.Bacc(
        "TRN2",
        target_bir_lowering=False,
        debug=False,
        enable_asserts=False,
        num_devices=_N_CORES,
    )
    enc_d = nc.dram_tensor("enc", [_BPC, _L, _D], dt.float32, kind="ExternalInput")
    w_d = nc.dram_tensor("w", [_D, _D], dt.float32, kind="ExternalInput")
    b_d = nc.dram_tensor("bias", [_D], dt.float32, kind="ExternalInput")
    out_d = nc.dram_tensor("out", [_T, _BPC, _L], dt.float32, kind="ExternalOutput")

    with tile.TileContext(nc) as tc:
        with (
            tc.tile_pool(name="const", bufs=1) as cpool,
            tc.tile_pool(name="encp", bufs=2) as encpool,
            tc.tile_pool(name="enc16p", bufs=2) as enc16pool,
            tc.tile_pool(name="etp", bufs=4) as etpool,
            tc.tile_pool(name="gtp", bufs=2) as gtpool,
            tc.tile_pool(name="big", bufs=3) as bigpool,
            tc.tile_pool(name="small", bufs=4) as smallpool,
            tc.tile_pool(name="ps_s", bufs=3, space="PSUM") as ps_s_pool,
            tc.tile_pool(name="ps_et", bufs=2, space="PSUM") as ps_et_pool,
        ):
            wm = cpool.tile([128, 512], dt.float16)
            nc.gpsimd.memset(wm[:], 0.25)

            ident_h = cpool.tile([128, 128], dt.float16)
            masks.make_identity(nc, ident_h[:])
            ident_f = cpool.tile([_D, _D], dt.float32)
            masks.make_identity(nc, ident_f[:])
            ident_i8 = cpool.tile([128, 128], dt.int8)
            masks.make_identity(nc, ident_i8[:])

            # --- W^T hi/lo (exact fp16 split), both halves at partitions 0-63
            w_sb = cpool.tile([_D, _D], dt.float32)
            nc.sync.dma_start(w_sb[:], w_d[:])
            ps_w = ps_s_pool.tile([_D, _D], dt.float32, tag="ps_s")
            nc.tensor.transpose(ps_w[:], w_sb[:], ident_f[:])
            w_hi = cpool.tile([_D, _D], dt.float16)
            nc.vector.tensor_copy(w_hi[:], ps_w[:])
            w_lo = cpool.tile([_D, _D], dt.float16)
            nc.vector.tensor_sub(w_lo[:], ps_w[:], w_hi[:])

            # --- b hi/lo as fp16 columns [64, 1] at partitions 0-63
            b_row = cpool.tile([1, _D], dt.float32)
            nc.sync.dma_start(b_row[:], b_d[:].unsqueeze(0))
            b_hi = cpool.tile([1, _D], dt.float16)
            nc.vector.tensor_copy(b_hi[:], b_row[:])
            b_lo = cpool.tile([1, _D], dt.float16)
            nc.vector.tensor_sub(b_lo[:], b_row[:], b_hi[:])
            ps_b = ps_et_pool.tile([_D, 4], dt.float16, tag="ps_et")
            nc.tensor.transpose(ps_b[:, 0:1], b_hi[:], ident_h[:1, :1])
            nc.tensor.transpose(ps_b[:, 2:3], b_lo[:], ident_h[:1, :1])
            b_cols = cpool.tile([_D, 4], dt.float16)
            nc.vector.tensor_copy(b_cols[:], ps_b[:])

            def prep_a(bb):
                """Load/cast/transpose/duplicate E for batch bb."""
                enc_sb = encpool.tile([128, 8 * _D], dt.float32, tag="enc")
                nc.sync.dma_start(
                    enc_sb[:].rearrange("p (n d) -> p n d", n=8),
                    enc_d[bb].rearrange("(n p) d -> p n d", p=128),
                )
                enc16 = enc16pool.tile([128, 8 * _D], dt.float16, tag="enc16")
                nc.vector.tensor_copy(enc16[:], enc_sb[:])
                # E16^T via 8 PE transposes, then duplicate rows via SBUF DMA:
                # et16d = [E16^T (p0-63); E16^T (p64-127)]
                ps_et16 = ps_et_pool.tile([_D, _L], dt.float16, tag="ps_et")
                for i in range(8):
                    nc.tensor.transpose(
                        ps_et16[:, i * 128 : (i + 1) * 128],
                        enc16[:, i * _D : (i + 1) * _D],
                        ident_h[:],
                    )
                et16d = etpool.tile([128, _L], dt.float16, tag="et16d")
                nc.vector.tensor_copy(et16d[: _D, :], ps_et16[:])
                nc.gpsimd.dma_start(et16d[_D :, :], et16d[: _D, :])
                return et16d

            def prep_eb(bb, et16d):
                """-E16.b for batch bb (K=64, uses only the base half)."""
                # c = E16 @ b per t-block: 8 tiny N=1 matmuls -> [128, 8]
                ps_eb = ps_et_pool.tile([128, 8], dt.float32, tag="ps_et")
                for i in range(8):
                    lhsT = et16d[: _D, i * 128 : (i + 1) * 128]
                    nc.tensor.matmul(
                        ps_eb[:, i : i + 1], lhsT, b_cols[:, 0:1],
                        start=True, stop=True,
                    )
                eb_neg = smallpool.tile([128, 8], dt.float32, tag="ebneg")
                nc.vector.tensor_scalar_mul(eb_neg[:], ps_eb[:], -1.0)
                return eb_neg

            def prep_g(bb, et16d):
                """G^T hi/lo for batch bb (K=64, no dup needed)."""
                # G^T = W @ E16^T (no bias) in PSUM f32 [64, 1024]
                ps_gt = ps_s_pool.tile([_D, _L], dt.float32, tag="ps_s")
                for c in range(2):
                    sl = slice(c * 512, (c + 1) * 512)
                    nc.tensor.matmul(
                        ps_gt[:, sl], w_hi[:], et16d[: _D, sl],
                        start=True, stop=False,
                    )
                    nc.tensor.matmul(
                        ps_gt[:, sl], w_lo[:], et16d[: _D, sl],
                        start=False, stop=True,
                    )
                # split: gt2 = [Glo (p0-63); Ghi (p64-127)], Ghi staged at p0-63
                gt_hi = gtpool.tile([_D, _L], dt.float16, tag="gthi")
                nc.scalar.activation(gt_hi[:], ps_gt[:], AF.Copy)
                gt2 = gtpool.tile([128, _L], dt.float16, tag="gt2")
                nc.vector.tensor_sub(gt2[: _D, :], ps_gt[:], gt_hi[:])
                nc.gpsimd.dma_start(gt2[_D :, :], gt_hi[:])
                return gt2

            def s_pair(bb, i2, et16d, gt2, eb_neg):
                """One pair of t-blocks -> one 1 MiB output DMA."""
                exp_sb = bigpool.tile([128, 2 * _L], dt.float32, tag="exp")
                sums = smallpool.tile([128, 2], dt.float32, tag="sums")
                recips = smallpool.tile([128, 2], dt.float32, tag="recips")
                for h in range(2):
                    i = 2 * i2 + h
                    bsl = slice(i * 128, (i + 1) * 128)
                    ps_s = ps_s_pool.tile([128, _L], dt.float32, tag="ps_s")
                    # diag chunk first so the mask overlaps the other matmul
                    c_diag = i // 4
                    for c in (c_diag, 1 - c_diag):
                        sl = slice(c * 512, (c + 1) * 512)
                        # ONE matmul per bank: [E16;E16] @ [Glo;Ghi] = E16@G
                        nc.tensor.matmul(
                            ps_s[:, sl], et16d[:, bsl], gt2[:, sl],
                            start=True, stop=True,
                        )
                        if c == c_diag:
                            # diagonal (j==t) <- -c_t (softmax-shift of diag 0)
                            nc.vector.copy_predicated(
                                ps_s[:, bsl],
                                ident_i8[:],
                                eb_neg[:, i : i + 1].to_broadcast([128, 128]),
                            )
                    nc.scalar.activation(
                        exp_sb[:, h * _L : (h + 1) * _L],
                        ps_s[:],
                        AF.Exp,
                        accum_out=sums[:, h : h + 1],
                    )
                nc.vector.reciprocal(recips[:], sums[:])
                for h in range(2):
                    nc.vector.tensor_scalar_mul(
                        exp_sb[:, h * _L : (h + 1) * _L],
                        exp_sb[:, h * _L : (h + 1) * _L],
                        recips[:, h : h + 1],
                    )
                dst = (
                    out_d[2 * i2 * 128 : (2 * i2 + 2) * 128, bb : bb + 1, :]
                    .squeeze(1)
                    .rearrange("(h p) j -> p h j", p=128)
                )
                nc.sync.dma_start(dst, exp_sb[:].rearrange("p (h j) -> p h j", h=2))

            # software-pipelined emission: all E-prep upfront, G/E.b prep
            # interleaved between the previous batch's S-pairs.
            et = [prep_a(bb) for bb in range(_BPC)]
            eb = [None] * _BPC
            gt = [None] * _BPC
            eb[0] = prep_eb(0, et[0])
            gt[0] = prep_g(0, et[0])
            # PE warm-up: dense matmuls fill the PE-idle window while the
            # first gt2 chain (ACT copy -> DVE sub -> dup DMA) completes,
            # un-throttling the HAM clock gate (4/8 -> 8/8) so the S matmuls
            # start at 2.4 GHz.
            ps_wm = ps_s_pool.tile([128, 512], dt.float32, tag="ps_s")
            for wi in range(10):
                nc.tensor.matmul(
                    ps_wm[:], wm[:, :128], wm[:],
                    start=(wi == 0), stop=(wi == 9),
                )
            for bb in range(_BPC):
                s_pair(bb, 0, et[bb], gt[bb], eb[bb])
                if bb + 1 < _BPC:
                    eb[bb + 1] = prep_eb(bb + 1, et[bb + 1])
                s_pair(bb, 1, et[bb], gt[bb], eb[bb])
                if bb + 1 < _BPC:
                    gt[bb + 1] = prep_g(bb + 1, et[bb + 1])
                s_pair(bb, 2, et[bb], gt[bb], eb[bb])
                s_pair(bb, 3, et[bb], gt[bb], eb[bb])

    nc.compile()
    _compiled_nc = nc
    return nc


def _numpy_fallback(enc, W, b, tl):
    energy = np.einsum("bld,ed->ble", enc, W) + b
    scores = np.einsum("bjd,btd->tbj", energy, enc[:, :tl, :])
    t_idx = np.arange(tl)
    scores[t_idx, :, t_idx] = 0.0
    m = scores.max(axis=-1, keepdims=True)
    e = np.exp(scores - m)
    return (e / e.sum(axis=-1, keepdims=True)).astype(np.float32)


def _run(encoder_outputs, W, b, target_length=1024, **run_kwargs):
    enc = np.ascontiguousarray(np.asarray(encoder_outputs, dtype=np.float32))
    Wn = np.ascontiguousarray(np.asarray(W, dtype=np.float32))
    bn = np.ascontiguousarray(np.asarray(b, dtype=np.float32))
    tl = int(target_length)
    if enc.shape != (_B, _L, _D) or tl != _T:
        return _numpy_fallback(enc, Wn, bn, tl), None

    from concourse.bass_utils import run_bass_kernel_spmd

    nc = _build()
    in_maps = [
        {"enc": enc[i * _BPC : (i + 1) * _BPC], "w": Wn, "bias": bn}
        for i in range(_N_CORES)
    ]
    res = run_bass_kernel_spmd(nc, in_maps, list(range(_N_CORES)), **run_kwargs)
    out = np.concatenate(
        [res.results[i]["out"] for i in range(_N_CORES)], axis=1
    ).astype(np.float32)
    return out, res


def kernel(encoder_outputs, W, b, target_length=1024):
    out, _ = _run(encoder_outputs, W, b, target_length)
    return out


def kernel_profiled(encoder_outputs, W, b, target_length=1024):
    """Run with NTFF tracing; returns (output, BassKernelResults)."""
    return _run(encoder_outputs, W, b, target_length, trace=True)


# revision 16
# speedup vs baseline: 1.0041x; 1.0041x over previous
"""Trainium2 Bass kernel for nn_Attn_45423574123081 (sparse_attention).

Computes, for inputs enc [B=32, L=1024, D=64], W [64, 64], b [64]:
    energy = enc @ W.T + b                       # [B, L, D]
    scores[t, b, j] = energy[b, j] . enc[b, t]   # [T=1024, B, L]
    scores[t, :, t] = 0
    out = softmax(scores, axis=-1)

Sharding: data-parallel over batch; 4 batches per core on 8 NeuronCores.

Key algebraic simplification: scores[t, :, j] = (E W^T E^T)[t, j] + c_t
with c_t = b . E[t] constant along the softmax axis j.  Softmax is
invariant to a per-row constant, so the bias only matters through the
zeroed diagonal: softmax(row with diag forced to 0) equals softmax of
the UNBIASED row with the diagonal forced to -c_t.  So G = E @ W^T
(no bias), and the diagonal write becomes copy_predicated(-E.b).

Precision strategy: matmuls run in fp16.  E is cast once to fp16
(~5e-4 score error).  W and G use an exact fp16 hi/lo split
(x = hi + lo), with the G halves stacked along the PE contraction axis
(K <= 128 is free on the systolic array), so each 512-wide PSUM bank of
S needs exactly ONE K=128 matmul:
    [E16; E16] @ [Glo; Ghi] = E16 @ G
Cross-partition placements (E16 row duplication, Ghi upper half) are
small SBUF->SBUF DMAs on the otherwise-idle GpSimd SWDGE queue, so they
never sit behind the megabyte output DMAs.

The emission is software-pipelined in two phases per batch (operand
prep A: load/cast/transpose/duplicate; prep B: G + E.b) interleaved
into the previous batch's S-block stream, so the per-engine in-order
queues never stall on a serial prep chain.

Softmax skips the max-subtraction: |scores| <= ~50 for this problem
family and exp(50) is far below f32 overflow.  exp runs on ScalarE with
a fused per-row accumulated sum; normalize is a per-partition scalar
multiply on VectorE.
"""

import numpy as np

_B, _L, _D, _T = 32, 1024, 64, 1024
_N_CORES = 8
_BPC = _B // _N_CORES  # batches per core

_compiled_nc = None


def _build():
    global _compiled_nc
    if _compiled_nc is not None:
        return _compiled_nc

    import concourse.bacc as bacc
    import concourse.mybir as mybir
    from concourse import tile, masks

    dt = mybir.dt
    AF = mybir.ActivationFunctionType

    nc = bacc.Bacc(
        "TRN2",
        target_bir_lowering=False,
        debug=False,
        enable_asserts=False,
        num_devices=_N_CORES,
    )
    enc_d = nc.dram_tensor("enc", [_BPC, _L, _D], dt.float32, kind="ExternalInput")
    w_d = nc.dram_tensor("w", [_D, _D], dt.float32, kind="ExternalInput")
    b_d = nc.dram_tensor("bias", [_D], dt.float32, kind="ExternalInput")
    out_d = nc.dram_tensor("out", [_T, _BPC, _L], dt.float32, kind="ExternalOutput")

    with tile.TileContext(nc) as tc:
        with (
            tc.tile_pool(name="const", bufs=1) as cpool,
            tc.tile_pool(name="encp", bufs=2) as encpool,
            tc.tile_pool(name="enc16p", bufs=2) as enc16pool,
            tc.tile_pool(name="etp", bufs=4) as etpool,
            tc.tile_pool(name="gtp", bufs=2) as gtpool,
            tc.tile_pool(name="big", bufs=3) as bigpool,
            tc.tile_pool(name="small", bufs=4) as smallpool,
            tc.tile_pool(name="ps_s", bufs=3, space="PSUM") as ps_s_pool,
            tc.tile_pool(name="ps_et", bufs=1, space="PSUM") as ps_et_pool,
            tc.tile_pool(name="ps_wm", bufs=1, space="PSUM") as ps_wm_pool,
        ):
            wm = cpool.tile([128, 512], dt.float16)
            nc.gpsimd.memset(wm[:], 0.25)

            ident_h = cpool.tile([128, 128], dt.float16)
            masks.make_identity(nc, ident_h[:])
            ident_f = cpool.tile([_D, _D], dt.float32)
            masks.make_identity(nc, ident_f[:])
            ident_i8 = cpool.tile([128, 128], dt.int8)
            masks.make_identity(nc, ident_i8[:])

            # --- W^T hi/lo (exact fp16 split), both halves at partitions 0-63
            w_sb = cpool.tile([_D, _D], dt.float32)
            nc.sync.dma_start(w_sb[:], w_d[:])
            ps_w = ps_s_pool.tile([_D, _D], dt.float32, tag="ps_s")
            nc.tensor.transpose(ps_w[:], w_sb[:], ident_f[:])
            w_hi = cpool.tile([_D, _D], dt.float16)
            nc.vector.tensor_copy(w_hi[:], ps_w[:])
            w_lo = cpool.tile([_D, _D], dt.float16)
            nc.vector.tensor_sub(w_lo[:], ps_w[:], w_hi[:])

            # --- b hi/lo as fp16 columns [64, 1] at partitions 0-63
            b_row = cpool.tile([1, _D], dt.float32)
            nc.sync.dma_start(b_row[:], b_d[:].unsqueeze(0))
            b_hi = cpool.tile([1, _D], dt.float16)
            nc.vector.tensor_copy(b_hi[:], b_row[:])
            b_lo = cpool.tile([1, _D], dt.float16)
            nc.vector.tensor_sub(b_lo[:], b_row[:], b_hi[:])
            ps_b = ps_et_pool.tile([_D, 4], dt.float16, tag="ps_et")
            nc.tensor.transpose(ps_b[:, 0:1], b_hi[:], ident_h[:1, :1])
            nc.tensor.transpose(ps_b[:, 2:3], b_lo[:], ident_h[:1, :1])
            b_cols = cpool.tile([_D, 4], dt.float16)
            nc.vector.tensor_copy(b_cols[:], ps_b[:])

            def prep_a(bb):
                """Load/cast/transpose/duplicate E for batch bb."""
                enc_sb = encpool.tile([128, 8 * _D], dt.float32, tag="enc")
                nc.sync.dma_start(
                    enc_sb[:].rearrange("p (n d) -> p n d", n=8),
                    enc_d[bb].rearrange("(n p) d -> p n d", p=128),
                )
                enc16 = enc16pool.tile([128, 8 * _D], dt.float16, tag="enc16")
                nc.gpsimd.tensor_copy(enc16[:], enc_sb[:])
                # E16^T via 8 PE transposes, then duplicate rows via SBUF DMA:
                # et16d = [E16^T (p0-63); E16^T (p64-127)]
                ps_et16 = ps_et_pool.tile([_D, _L], dt.float16, tag="ps_et")
                for i in range(8):
                    nc.tensor.transpose(
                        ps_et16[:, i * 128 : (i + 1) * 128],
                        enc16[:, i * _D : (i + 1) * _D],
                        ident_h[:],
                    )
                et16d = etpool.tile([128, _L], dt.float16, tag="et16d")
                nc.vector.tensor_copy(et16d[: _D, :], ps_et16[:])
                nc.gpsimd.dma_start(et16d[_D :, :], et16d[: _D, :])
                return et16d

            def prep_eb(bb, et16d):
                """-E16.b for batch bb (K=64, uses only the base half)."""
                # c = E16 @ b per t-block: 8 tiny N=1 matmuls -> [128, 8]
                ps_eb = ps_et_pool.tile([128, 8], dt.float32, tag="ps_et")
                for i in range(8):
                    lhsT = et16d[: _D, i * 128 : (i + 1) * 128]
                    nc.tensor.matmul(
                        ps_eb[:, i : i + 1], lhsT, b_cols[:, 0:1],
                        start=True, stop=True,
                    )
                eb_neg = smallpool.tile([128, 8], dt.float32, tag="ebneg")
                nc.vector.tensor_scalar_mul(eb_neg[:], ps_eb[:], -1.0)
                return eb_neg

            def prep_g(bb, et16d):
                """G^T hi/lo for batch bb (K=64, no dup needed)."""
                # G^T = W @ E16^T (no bias) in PSUM f32 [64, 1024]
                ps_gt = ps_s_pool.tile([_D, _L], dt.float32, tag="ps_s")
                for c in range(2):
                    sl = slice(c * 512, (c + 1) * 512)
                    nc.tensor.matmul(
                        ps_gt[:, sl], w_hi[:], et16d[: _D, sl],
                        start=True, stop=False,
                    )
                    nc.tensor.matmul(
                        ps_gt[:, sl], w_lo[:], et16d[: _D, sl],
                        start=False, stop=True,
                    )
                # split: gt2 = [Glo (p0-63); Ghi (p64-127)], Ghi staged at p0-63
                gt_hi = gtpool.tile([_D, _L], dt.float16, tag="gthi")
                nc.scalar.activation(gt_hi[:], ps_gt[:], AF.Copy)
                gt2 = gtpool.tile([128, _L], dt.float16, tag="gt2")
                nc.vector.tensor_sub(gt2[: _D, :], ps_gt[:], gt_hi[:])
                nc.gpsimd.dma_start(gt2[_D :, :], gt_hi[:])
                return gt2

            def s_pair(bb, i2, et16d, gt2, eb_neg, split_dma=False):
                """One pair of t-blocks -> one 1 MiB output DMA."""
                exp_sb = bigpool.tile([128, 2 * _L], dt.float32, tag="exp")
                sums = smallpool.tile([128, 2], dt.float32, tag="sums")
                recips = smallpool.tile([128, 2], dt.float32, tag="recips")
                for h in range(2):
                    i = 2 * i2 + h
                    bsl = slice(i * 128, (i + 1) * 128)
                    ps_s = ps_s_pool.tile([128, _L], dt.float32, tag="ps_s")
                    # diag chunk first so the mask overlaps the other matmul
                    c_diag = i // 4
                    for c in (c_diag, 1 - c_diag):
                        sl = slice(c * 512, (c + 1) * 512)
                        # ONE matmul per bank: [E16;E16] @ [Glo;Ghi] = E16@G
                        nc.tensor.matmul(
                            ps_s[:, sl], et16d[:, bsl], gt2[:, sl],
                            start=True, stop=True,
                        )
                        if c == c_diag:
                            # diagonal (j==t) <- -c_t (softmax-shift of diag 0)
                            nc.vector.copy_predicated(
                                ps_s[:, bsl],
                                ident_i8[:],
                                eb_neg[:, i : i + 1].to_broadcast([128, 128]),
                            )
                    nc.scalar.activation(
                        exp_sb[:, h * _L : (h + 1) * _L],
                        ps_s[:],
                        AF.Exp,
                        accum_out=sums[:, h : h + 1],
                    )
                # HAM keep-warm filler: dense PE work so the activity
                # monitor never re-throttles the clock to 1.2 GHz
                ps_f = ps_wm_pool.tile([128, 256], dt.float32, tag="wm")
                for wi in range(2):
                    nc.tensor.matmul(
                        ps_f[:], wm[:, :128], wm[:, :256],
                        start=(wi == 0), stop=(wi == 1),
                    )
                nc.vector.reciprocal(recips[:], sums[:])
                for h in range(2):
                    nc.vector.tensor_scalar_mul(
                        exp_sb[:, h * _L : (h + 1) * _L],
                        exp_sb[:, h * _L : (h + 1) * _L],
                        recips[:, h : h + 1],
                    )
                dst = (
                    out_d[2 * i2 * 128 : (2 * i2 + 2) * 128, bb : bb + 1, :]
                    .squeeze(1)
                    .rearrange("(h p) j -> p h j", p=128)
                )
                src_v = exp_sb[:].rearrange("p (h j) -> p h j", h=2)
                if split_dma:
                    # tail latency: ship each t-block as soon as it is ready
                    nc.sync.dma_start(dst[:, 0, :], src_v[:, 0, :])
                    nc.sync.dma_start(dst[:, 1, :], src_v[:, 1, :])
                else:
                    nc.sync.dma_start(dst, src_v)

            # software-pipelined emission: all E-prep upfront, G/E.b prep
            # interleaved between the previous batch's S-pairs.
            et = [prep_a(bb) for bb in range(_BPC)]
            eb = [None] * _BPC
            gt = [None] * _BPC
            eb[0] = prep_eb(0, et[0])
            gt[0] = prep_g(0, et[0])
            # PE warm-up: dense matmuls fill the PE-idle window while the
            # first gt2 chain (ACT copy -> DVE sub -> dup DMA) completes,
            # un-throttling the HAM clock gate (4/8 -> 8/8) so the S matmuls
            # start at 2.4 GHz.
            ps_wm = ps_wm_pool.tile([128, 256], dt.float32, tag="wm")
            for wi in range(20):
                nc.tensor.matmul(
                    ps_wm[:], wm[:, :128], wm[:, :256],
                    start=(wi == 0), stop=(wi == 19),
                )
            for bb in range(_BPC):
                s_pair(bb, 0, et[bb], gt[bb], eb[bb])
                if bb + 1 < _BPC:
                    eb[bb + 1] = prep_eb(bb + 1, et[bb + 1])
                s_pair(bb, 1, et[bb], gt[bb], eb[bb])
                if bb + 1 < _BPC:
                    gt[bb + 1] = prep_g(bb + 1, et[bb + 1])
                s_pair(bb, 2, et[bb], gt[bb], eb[bb])
                s_pair(bb, 3, et[bb], gt[bb], eb[bb],
                       split_dma=(bb == _BPC - 1))

    nc.compile()
    _compiled_nc = nc
    return nc


def _numpy_fallback(enc, W, b, tl):
    energy = np.einsum("bld,ed->ble", enc, W) + b
    scores = np.einsum("bjd,btd->tbj", energy, enc[:, :tl, :])
    t_idx = np.arange(tl)
    scores[t_idx, :, t_idx] = 0.0
    m = scores.max(axis=-1, keepdims=True)
    e = np.exp(scores - m)
    return (e / e.sum(axis=-1, keepdims=True)).astype(np.float32)


def _run(encoder_outputs, W, b, target_length=1024, **run_kwargs):
    enc = np.ascontiguousarray(np.asarray(encoder_outputs, dtype=np.float32))
    Wn = np.ascontiguousarray(np.asarray(W, dtype=np.float32))
    bn = np.ascontiguousarray(np.asarray(b, dtype=np.float32))
    tl = int(target_length)
    if enc.shape != (_B, _L, _D) or tl != _T:
        return _numpy_fallback(enc, Wn, bn, tl), None

    from concourse.bass_utils import run_bass_kernel_spmd

    nc = _build()
    in_maps = [
        {"enc": enc[i * _BPC : (i + 1) * _BPC], "w": Wn, "bias": bn}
        for i in range(_N_CORES)
    ]
    res = run_bass_kernel_spmd(nc, in_maps, list(range(_N_CORES)), **run_kwargs)
    out = np.concatenate(
        [res.results[i]["out"] for i in range(_N_CORES)], axis=1
    ).astype(np.float32)
    return out, res


def kernel(encoder_outputs, W, b, target_length=1024):
    out, _ = _run(encoder_outputs, W, b, target_length)
    return out


def kernel_profiled(encoder_outputs, W, b, target_length=1024):
    """Run with NTFF tracing; returns (output, BassKernelResults)."""
    return _run(encoder_outputs, W, b, target_length, trace=True)
